# revision 30
# baseline (speedup 1.0000x reference)
"""Trainium2 Bass kernel for nn_BayesLSTMModel (2-layer Bayes-LSTM + decoder).

Sharding: 8 cores SPMD.
 - Recurrence: tensor-parallel over the hidden dim. Core c owns hidden dims
   [c*128, (c+1)*128) of BOTH layers; computes the 8 gate tiles (i,f,o,ib,fb,
   ob,g,gb) for its slice each step. Two per-step AllGathers exchange bf16 h
   slices: AG1 carries h1[s] right after layer-1's step, AG2 carries
   h2[s-6] after layer-2's (lagged) step, so each collective overlaps the
   other layer's matmuls.
 - Layer 2 runs 6 supersteps behind layer 1; its input-side gates
   (Wx2 @ o0[t]) are batched over 4-step groups (N=128 matmuls) instead of
   per-step N=32 matmuls.
 - Gates layout: [gate-dim partition, batch free] -> weights are the matmul
   stationary (bf16, FWL), rhs is gathered h^T [128,32]; biases are folded
   into the batched input-gate copies (per-partition ACT bias); no
   transposes anywhere on device (host pre-transposes everything).
 - L1 input gates (emb @ Wx1^T): chunk 0 in a prologue, remaining chunks
   interleaved into the superstep loop, staged through DRAM.
 - Decoder: vocab-parallel, one quarter-group (2x 128x500 output tiles) per
   superstep, filling the AllGather bubbles.
Host side: embedding gather, weight re-layout/transpose, softmax(mix),
output reassembly.
"""

import os
import sys

sys.path.insert(0, "/opt/trn_rl_repo")

import numpy as np
import ml_dtypes

import concourse.bass as bass
import concourse.bacc as bacc
import concourse.mybir as mybir
import concourse.tile as tile
from concourse.bass_utils import run_bass_kernel_spmd

BF16 = ml_dtypes.bfloat16
NC = 8           # cores
B = 32           # batch
H = 1024         # hidden
NINP = 1024
V = 32000
HS = H // NC     # 128: per-core hidden slice
VS = V // NC     # 4000: per-core vocab slice
KT = H // 128    # 8 k-tiles of the contraction dim
D2 = 6           # layer-2 lag in supersteps

# m-tile order: sigmoid group first (i, f, o, ib, fb, ob), then tanh (g, gb).
# (gate_index t in reference split order i=0 f=1 g=2 o=3, is_bayes)
M_ORDER = [(0, 0), (1, 0), (3, 0), (0, 1), (1, 1), (3, 1), (2, 0), (2, 1)]
# columns in the [128, 256] gate tile: m-tile m -> cols [32m, 32m+32)
# after mixing ([128, 128] tile): [ig | fg | og | gg]

f32 = mybir.dt.float32
bf16 = mybir.dt.bfloat16


def build_program(S, sim_mode=False):
    """Build the SPMD Bass program for sequence length S. Returns compiled nc.

    sim_mode: single-core, collectives replaced by a local DMA (numerically
    wrong, timing structure similar) -- for TimelineSim analysis.
    """
    n_tok = S * B
    TCH = min(512, n_tok)          # tokens per Gx1 chunk
    n_ch = n_tok // TCH
    steps_per_ch = TCH // B

    nc = bacc.Bacc(
        "TRN2", target_bir_lowering=False, debug=False,
        num_devices=1 if sim_mode else NC,
    )

    wr_in = nc.dram_tensor("wr", [2, 128, KT * 8 * 128], bf16, kind="ExternalInput")
    wx_in = nc.dram_tensor("wx", [2, 128, KT * 8 * 128], bf16, kind="ExternalInput")
    decw_in = nc.dram_tensor("decw", [128, KT, VS], bf16, kind="ExternalInput")
    decb_in = nc.dram_tensor("decb", [1, VS], bf16, kind="ExternalInput")
    embt_in = nc.dram_tensor("embt", [KT, 128, n_tok], bf16, kind="ExternalInput")
    h0t1_in = nc.dram_tensor("h0t1", [128, KT, B], bf16, kind="ExternalInput")
    h0t2_in = nc.dram_tensor("h0t2", [128, KT, B], bf16, kind="ExternalInput")
    c0_in = nc.dram_tensor("c0own", [2, 128, B], f32, kind="ExternalInput")
    bias_in = nc.dram_tensor("bias", [128, 8, 2], f32, kind="ExternalInput")
    probs_in = nc.dram_tensor("probs", [128, 8, 2], f32, kind="ExternalInput")

    dec_out = nc.dram_tensor("dec", [n_tok, VS], f32, kind="ExternalOutput")
    hids_out = nc.dram_tensor("hids", [2, 128, B], f32, kind="ExternalOutput")
    cells_out = nc.dram_tensor("cells", [2, 128, B], f32, kind="ExternalOutput")

    n_groups = n_tok // 128        # decoder token groups of 128
    sig_f = mybir.ActivationFunctionType.Sigmoid
    tanh_f = mybir.ActivationFunctionType.Tanh
    copy_f = mybir.ActivationFunctionType.Copy
    ident_f = mybir.ActivationFunctionType.Identity

    with tile.TileContext(nc) as tc:
        with (
            tc.tile_pool(name="wpool", bufs=1) as wpool,
            tc.tile_pool(name="state", bufs=1) as state,
            tc.tile_pool(name="ring", bufs=3) as ring,
            tc.tile_pool(name="gxring", bufs=3) as gxring,
            tc.tile_pool(name="acts", bufs=2) as acts,
            tc.tile_pool(name="o1pool", bufs=2) as o1pool,
            tc.tile_pool(name="decstg", bufs=2) as decstg,
            tc.tile_pool(name="ps1pool", bufs=2, space="PSUM") as ps1pool,
            tc.tile_pool(name="ps2pool", bufs=2, space="PSUM") as ps2pool,
            tc.tile_pool(name="bigps", bufs=4, space="PSUM") as bigps,
            tc.tile_pool(name="dram", bufs=1, space="DRAM") as dram,
            tc.tile_pool(name="agdram", bufs=3, space="DRAM") as agdram,
        ):
            # ---- resident weights & constants ----
            wr_sb = wpool.tile([128, 2, KT, 8, 128], bf16)
            nc.sync.dma_start(wr_sb[:], wr_in.ap().rearrange("l p x -> p l x"))
            wx_sb = wpool.tile([128, 2, KT, 8, 128], bf16)
            nc.sync.dma_start(wx_sb[:], wx_in.ap().rearrange("l p x -> p l x"))
            decw_sb = wpool.tile([128, KT, VS], bf16)
            nc.sync.dma_start(decw_sb[:], decw_in[:])
            decb_sb = wpool.tile([1, VS], bf16)
            nc.sync.dma_start(decb_sb[:], decb_in[:])
            bias_sb = wpool.tile([128, 8, 2], f32)
            nc.sync.dma_start(bias_sb[:], bias_in[:])
            probs_sb = wpool.tile([128, 8, 2], f32)
            nc.sync.dma_start(probs_sb[:], probs_in[:])
            ones_sb = wpool.tile([1, 128], bf16)
            nc.vector.memset(ones_sb[:], 1.0)

            c_st = state.tile([128, 2, B], f32)   # cell state, both layers
            nc.sync.dma_start(c_st[:], c0_in.ap().rearrange("l p b -> p l b"))

            gx1_dram = dram.tile([128, 8, n_tok], f32)  # [p, m, tok]

            # Warm the recurrence PSUM slots: one start=True matmul writing
            # zeros sets every has_written bit, so steady-state groups can
            # begin with start=False and accumulate onto gx data placed in
            # PSUM by DMA/ACT-copy.
            zero_rhs = wpool.tile([1, 256], bf16)
            nc.vector.memset(zero_rhs[:], 0.0)
            for pool, tagn in ((ps1pool, "ps1"), (ps2pool, "ps2")):
                for _ in range(2):
                    warm = pool.tile([128, 256], f32, tag=tagn, name="warm")
                    nc.tensor.matmul(warm[:], ones_sb[:], zero_rhs[:],
                                     start=True, stop=True)

            # ---- Gx1 chunk: batched emb @ Wx1^T with bias folded in ----
            def gx1_chunk(ch, ms):
                if ms == 0:
                    rhse = ring.tile([128, KT, TCH], bf16, tag="rhse", bufs=2)
                    gx1_chunk.rhse = rhse
                    nc.sync.dma_start(
                        rhse[:],
                        embt_in[:, :, ch * TCH:(ch + 1) * TCH].rearrange(
                            "k p t -> p k t"),
                    )
                rhse = gx1_chunk.rhse
                for m in (2 * ms, 2 * ms + 1):
                    psg = bigps.tile([128, TCH], f32, tag="bigps")
                    for k in range(KT):
                        nc.tensor.matmul(
                            psg[:], wx_sb[:, 0, k, m, :], rhse[:, k, :],
                            start=(k == 0), stop=(k == KT - 1),
                        )
                    stg = decstg.tile([128, TCH], f32, tag="gxstg", bufs=2)
                    nc.scalar.activation(stg[:], psg[:], ident_f,
                                         bias=bias_sb[:, m:m + 1, 0])
                    nc.sync.dma_start(
                        gx1_dram[:, m, ch * TCH:(ch + 1) * TCH], stg[:])

            for ms in range(4):
                gx1_chunk(0, ms)

            # ---- per-step act chain (bias already folded into gx) ----
            def act_chain(l, gsum, h_bf, emit_fp32_h):
                """gsum [128,256] f32 -> h (bf16) into h_bf [128,32];
                updates c_st[:, l, :]."""
                sig = acts.tile([128, 256], f32, tag=f"sig{l}")
                nc.scalar.activation(sig[:, 0:192], gsum[:, 0:192], sig_f)
                nc.scalar.activation(sig[:, 192:256], gsum[:, 192:256], tanh_f)
                mixed = acts.tile([128, 256], f32, tag=f"mix{l}")
                for m in range(8):
                    nc.vector.tensor_scalar_mul(
                        mixed[:, m * 32:(m + 1) * 32],
                        sig[:, m * 32:(m + 1) * 32],
                        probs_sb[:, m:m + 1, l],
                    )
                gval = acts.tile([128, 128], f32, tag=f"gval{l}")
                nc.vector.tensor_tensor(
                    gval[:, 0:96], mixed[:, 0:96], mixed[:, 96:192],
                    mybir.AluOpType.add)
                nc.vector.tensor_tensor(
                    gval[:, 96:128], mixed[:, 192:224], mixed[:, 224:256],
                    mybir.AluOpType.add)
                t1 = acts.tile([128, B], f32, tag=f"t1{l}")
                nc.vector.tensor_tensor(
                    t1[:], gval[:, 32:64], c_st[:, l, :], mybir.AluOpType.mult)
                t2 = acts.tile([128, B], f32, tag=f"t2{l}")
                nc.vector.tensor_tensor(
                    t2[:], gval[:, 0:32], gval[:, 96:128], mybir.AluOpType.mult)
                nc.vector.tensor_tensor(
                    c_st[:, l, :], t1[:], t2[:], mybir.AluOpType.add)
                tanhc = acts.tile([128, B], f32, tag=f"tanhc{l}")
                nc.scalar.activation(tanhc[:], c_st[:, l, :], tanh_f)
                nc.vector.tensor_tensor(
                    h_bf[:], gval[:, 64:96], tanhc[:], mybir.AluOpType.mult)
                if emit_fp32_h:
                    hf = acts.tile([128, B], f32, tag="hfinal")
                    nc.vector.tensor_tensor(
                        hf[:], gval[:, 64:96], tanhc[:], mybir.AluOpType.mult)
                    nc.sync.dma_start(hids_out[l], hf[:])

            def allgather(h_bf, tag):
                bounce = agdram.tile([128, B], bf16, tag=f"agin{tag}")
                nc.sync.dma_start(bounce[:], h_bf[:])
                ag = agdram.tile([128 * NC, B], bf16, tag=f"agout{tag}",
                                 addr_space="Local" if sim_mode else "Shared")
                if sim_mode:
                    nc.sync.dma_start(ag[0:128, :], bounce[:])
                else:
                    nc.gpsimd.collective_compute(
                        "AllGather", mybir.AluOpType.bypass,
                        replica_groups=[list(range(NC))],
                        ins=[bounce.opt()], outs=[ag.opt()],
                    )
                return ag

            def decode_quarter(g, q, o1g):
                w = VS // 8
                for ch in range(2):
                    col0 = (q * 2 + ch) * w
                    psd = bigps.tile([128, w], f32, tag="bigps")
                    for k in range(KT):
                        nc.tensor.matmul(
                            psd[:, 0:w], o1g[:, k, :],
                            decw_sb[:, k, col0:col0 + w],
                            start=(k == 0), stop=False,
                        )
                    nc.tensor.matmul(
                        psd[:, 0:w], ones_sb[:], decb_sb[:, col0:col0 + w],
                        start=False, stop=True,
                    )
                    stg = decstg.tile([128, w], f32, tag="decstg")
                    nc.scalar.activation(stg[:, 0:w], psd[:, 0:w], copy_f)
                    nc.sync.dma_start(
                        dec_out[g * 128:(g + 1) * 128, col0:col0 + w],
                        stg[:, 0:w])

            o0_tiles = {}
            o1_tiles = {}
            gx2_tiles = {}
            ag1 = ag2 = None
            for s in range(S + D2 + 1):
                # ---- gathered h^T tiles for this superstep ----
                if s == 0:
                    ht1 = ring.tile([128, KT, B], bf16, tag="ht1")
                    nc.sync.dma_start(ht1[:], h0t1_in[:])
                elif s <= S:
                    ht1 = ring.tile([128, KT, B], bf16, tag="ht1")
                    nc.sync.dma_start(
                        ht1[:], ag1.rearrange("(k p) b -> p k b", p=128))
                if s == D2:
                    ht2 = ring.tile([128, KT, B], bf16, tag="ht2")
                    nc.sync.dma_start(ht2[:], h0t2_in[:])
                elif s >= D2 + 1:
                    ht2 = ring.tile([128, KT, B], bf16, tag="ht2")
                    nc.sync.dma_start(
                        ht2[:], ag2.rearrange("(k p) b -> p k b", p=128))

                # ---- o0 history (h1) for Gx2 batching ----
                if 1 <= s <= S:
                    t1s = s - 1
                    g, slot = t1s // 4, t1s % 4
                    if slot == 0:
                        o0g_t = o1pool.tile([128, KT, 128], bf16, tag="o0g", name="o0g_t")
                        o0_tiles[g] = o0g_t
                    nc.scalar.copy(
                        o0_tiles[g][:, :, slot * 32:(slot + 1) * 32], ht1[:])
                # ---- o1 history (h2) for the decoder ----
                if s >= D2 + 1:
                    t2s = s - D2 - 1
                    g, slot = t2s // 4, t2s % 4
                    if slot == 0:
                        o1g_t = o1pool.tile([128, KT, 128], bf16, tag="o1g", name="o1g_t")
                        o1_tiles[g] = o1g_t
                    nc.scalar.copy(
                        o1_tiles[g][:, :, slot * 32:(slot + 1) * 32], ht2[:])

                # ---- interleaved Gx1 chunks ----
                if s % 16 < 4 and s // 16 + 1 < n_ch:
                    gx1_chunk(s // 16 + 1, s % 16)

                # ---- Gx2 batch for group g at s = 4g+5 ----
                if s >= 5 and (s - 5) % 4 == 0 and (s - 5) // 4 < S // 4:
                    g = (s - 5) // 4
                    gx2g = gxring.tile([128, 8, 128], f32, tag="gx2g", bufs=2)
                    gx2_tiles[g] = gx2g  # noqa
                    for half in range(2):
                        psb = bigps.tile([128, 512], f32, tag="bigps")
                        for mi in range(4):
                            m = half * 4 + mi
                            for k in range(KT):
                                nc.tensor.matmul(
                                    psb[:, mi * 128:(mi + 1) * 128],
                                    wx_sb[:, 1, k, m, :],
                                    o0_tiles[g][:, k, :],
                                    start=(k == 0), stop=(k == KT - 1),
                                )
                        for mi in range(4):
                            m = half * 4 + mi
                            nc.scalar.activation(
                                gx2g[:, m, :],
                                psb[:, mi * 128:(mi + 1) * 128],
                                ident_f, bias=bias_sb[:, m:m + 1, 1])

                # ---- L1 step s ----
                if s < S:
                    gx = gxring.tile([128, 8, B], f32, tag="gx")
                    nc.sync.dma_start(
                        gx[:], gx1_dram[:, :, s * B:(s + 1) * B])
                    ps1 = ps1pool.tile([128, 256], f32, tag="ps1")
                    nc.scalar.copy(
                        ps1.rearrange("p (m b) -> p m b", b=B), gx[:])
                    for m in range(8):
                        for k in range(KT):
                            nc.tensor.matmul(
                                ps1[:, m * 32:(m + 1) * 32],
                                wr_sb[:, 0, k, m, :], ht1[:, k, :],
                                start=False, stop=(k == KT - 1),
                                skip_group_check=True,
                            )
                    h1_bf = acts.tile([128, B], bf16, tag="h1bf")
                    act_chain(0, ps1, h1_bf, emit_fp32_h=(s == S - 1))
                    ag1 = allgather(h1_bf, 1)

                # ---- L2 step s - D2 ----
                if D2 <= s < S + D2:
                    t2 = s - D2
                    ps2 = ps2pool.tile([128, 256], f32, tag="ps2")
                    nc.scalar.copy(
                        ps2.rearrange("p (m b) -> p m b", b=B),
                        gx2_tiles[t2 // 4][:, :, (t2 % 4) * 32:(t2 % 4 + 1) * 32])
                    for m in range(8):
                        for k in range(KT):
                            nc.tensor.matmul(
                                ps2[:, m * 32:(m + 1) * 32],
                                wr_sb[:, 1, k, m, :], ht2[:, k, :],
                                start=False, stop=(k == KT - 1),
                                skip_group_check=True,
                            )
                    h2_bf = acts.tile([128, B], bf16, tag="h2bf")
                    act_chain(1, ps2, h2_bf, emit_fp32_h=(s == S + D2 - 1))
                    ag2 = allgather(h2_bf, 2)

                # ---- decoder: one quarter-group per superstep once ready ----
                for q in range(4):
                    gq, rem = divmod(s - (D2 + 4 + q), 4)
                    if rem == 0 and 0 <= gq < n_groups - 1:
                        decode_quarter(gq, q, o1_tiles[gq])

            # ---- epilogue: final decoder group ----
            gq = n_groups - 1
            for q in range(4):
                decode_quarter(gq, q, o1_tiles[gq])

            # ---- state outputs ----
            for l in range(2):
                nc.sync.dma_start(cells_out[l], c_st[:, l, :])

    nc.compile()
    return nc


_prog_cache = {}


def _get_program(S):
    if S not in _prog_cache:
        _prog_cache[S] = build_program(S)
    return _prog_cache[S]


def _prep_inputs(S, x, h0, c0, emb_W, dec_W, dec_b, W_ih, b_ih, W_hh, Wb, bb, mix):
    """Host-side prep: returns in_maps (list of 8 dicts)."""
    n_tok = S * B
    x = np.asarray(x).reshape(-1)
    emb = np.asarray(emb_W)[x]                      # [n_tok, NINP]
    embt = np.ascontiguousarray(emb.T).astype(BF16).reshape(KT, 128, n_tok)

    mix = np.asarray(mix)
    probs = np.exp(mix - np.max(mix, -1, keepdims=True))
    probs = (probs / probs.sum(-1, keepdims=True)).astype(np.float32)  # [2,4,2]

    Wb = np.asarray(Wb)
    W_ih = np.asarray(W_ih)
    W_hh = np.asarray(W_hh)
    b_ih = np.asarray(b_ih)
    bb = np.asarray(bb)
    h0 = np.asarray(h0)
    c0 = np.asarray(c0)
    dec_W = np.asarray(dec_W)
    dec_b = np.asarray(dec_b)

    # per-layer big matrices, rows ordered by m-tile blocks of H rows
    def stack_rows(l, part):  # part: 0 = x-side [.., :NINP], 1 = h-side
        blocks = []
        for t, is_bay in M_ORDER:
            if is_bay:
                w = Wb[l][t][:, NINP:] if part else Wb[l][t][:, :NINP]
            else:
                w = W_hh[l][t * H:(t + 1) * H] if part else W_ih[l][t * H:(t + 1) * H]
            blocks.append(w)
        return np.stack(blocks)                      # [8, H, 1024]

    wx_all = [stack_rows(l, 0) for l in range(2)]
    wr_all = [stack_rows(l, 1) for l in range(2)]

    def core_w(wlist, c):
        # -> [2, 128, KT*8*128]: [l, p, k*1024 + m*128 + j]
        out = np.empty((2, 128, KT * 8 * 128), BF16)
        for l in range(2):
            a = wlist[l][:, c * HS:(c + 1) * HS, :]   # [8m, 128j, 1024K]
            a = a.reshape(8, HS, KT, 128)             # [m, j, k, p]
            out[l] = a.transpose(3, 2, 0, 1).reshape(128, -1).astype(BF16)
        return out

    def core_bias(c):
        out = np.zeros((128, 8, 2), np.float32)
        for l in range(2):
            for m, (t, is_bay) in enumerate(M_ORDER):
                if is_bay:
                    out[:, m, l] = bb[l][t][c * HS:(c + 1) * HS]
                else:
                    out[:, m, l] = 2.0 * b_ih[l][t * H + c * HS:t * H + (c + 1) * HS]
        return out

    def core_probs():
        out = np.zeros((128, 8, 2), np.float32)
        for l in range(2):
            for m, (t, is_bay) in enumerate(M_ORDER):
                out[:, m, l] = probs[l, t, 1 if is_bay else 0]
        return out

    h0t1 = h0[0].T.reshape(KT, 128, B).transpose(1, 0, 2).astype(BF16)
    h0t2 = h0[1].T.reshape(KT, 128, B).transpose(1, 0, 2).astype(BF16)

    probs_arr = core_probs()
    in_maps = []
    for c in range(NC):
        dwt = dec_W[c * VS:(c + 1) * VS].T           # [1024, VS]
        decw = np.ascontiguousarray(dwt).astype(BF16).reshape(KT, 128, VS)
        decw = np.ascontiguousarray(decw.transpose(1, 0, 2))
        in_maps.append(
            dict(
                wr=core_w(wr_all, c),
                wx=core_w(wx_all, c),
                decw=decw,
                decb=dec_b[c * VS:(c + 1) * VS].reshape(1, VS).astype(BF16),
                embt=embt,
                h0t1=np.ascontiguousarray(h0t1),
                h0t2=np.ascontiguousarray(h0t2),
                c0own=np.ascontiguousarray(
                    np.stack([c0[l][:, c * HS:(c + 1) * HS].T for l in range(2)])
                ).astype(np.float32),
                bias=core_bias(c),
                probs=probs_arr,
            )
        )
    return in_maps


def run(S, inputs):
    nc = _get_program(S)
    in_maps = _prep_inputs(S, **inputs)
    res = run_bass_kernel_spmd(nc, in_maps, core_ids=list(range(NC)))
    dec = np.concatenate(
        [r["dec"].reshape(S, B, VS) for r in res.results], axis=2)
    hids = np.concatenate(
        [r["hids"].transpose(0, 2, 1) for r in res.results], axis=2)
    cells = np.concatenate(
        [r["cells"].transpose(0, 2, 1) for r in res.results], axis=2)
    return dec, hids, cells


def kernel(**inputs):
    S = np.asarray(inputs["x"]).shape[0]
    return run(S, inputs)


# revision 33
# speedup vs baseline: 2.0295x; 2.0295x over previous
"""Trainium2 Bass kernel for nn_BayesLSTMModel (2-layer Bayes-LSTM + decoder).

Sharding: 8 cores SPMD.
 - Recurrence: tensor-parallel over the hidden dim. Core c owns hidden dims
   [c*128, (c+1)*128) of BOTH layers; computes the 8 gate tiles (i,f,o,ib,fb,
   ob,g,gb) for its slice each step. Two per-step AllGathers exchange bf16 h
   slices: AG1 carries h1[s] right after layer-1's step, AG2 carries
   h2[s-6] after layer-2's (lagged) step, so each collective overlaps the
   other layer's matmuls.
 - Layer 2 runs 6 supersteps behind layer 1; its input-side gates
   (Wx2 @ o0[t]) are batched over 4-step groups (N=128 matmuls) instead of
   per-step N=32 matmuls.
 - Gates layout: [gate-dim partition, batch free] -> weights are the matmul
   stationary (bf16, FWL), rhs is gathered h^T [128,32]; biases are folded
   into the batched input-gate copies (per-partition ACT bias); no
   transposes anywhere on device (host pre-transposes everything).
 - L1 input gates (emb @ Wx1^T): chunk 0 in a prologue, remaining chunks
   interleaved into the superstep loop, staged through DRAM.
 - Decoder: vocab-parallel, one quarter-group (2x 128x500 output tiles) per
   superstep, filling the AllGather bubbles.
Host side: embedding gather, weight re-layout/transpose, softmax(mix),
output reassembly.
"""

import os
import sys

sys.path.insert(0, "/opt/trn_rl_repo")

import numpy as np
import ml_dtypes

import concourse.bass as bass
import concourse.bacc as bacc
import concourse.mybir as mybir
import concourse.tile as tile
from concourse.bass_utils import run_bass_kernel_spmd

BF16 = ml_dtypes.bfloat16
NC = 8           # cores
B = 32           # batch
H = 1024         # hidden
NINP = 1024
V = 32000
HS = H // NC     # 128: per-core hidden slice
VS = V // NC     # 4000: per-core vocab slice
KT = H // 128    # 8 k-tiles of the contraction dim
D2 = 6           # layer-2 lag in supersteps

# m-tile order: sigmoid group first (i, f, o, ib, fb, ob), then tanh (g, gb).
# (gate_index t in reference split order i=0 f=1 g=2 o=3, is_bayes)
M_ORDER = [(0, 0), (1, 0), (3, 0), (0, 1), (1, 1), (3, 1), (2, 0), (2, 1)]
# columns in the [128, 256] gate tile: m-tile m -> cols [32m, 32m+32)
# after mixing ([128, 128] tile): [ig | fg | og | gg]

f32 = mybir.dt.float32
bf16 = mybir.dt.bfloat16


def build_program(S, sim_mode=False):
    """Build the SPMD Bass program for sequence length S. Returns compiled nc.

    sim_mode: single-core, collectives replaced by a local DMA (numerically
    wrong, timing structure similar) -- for TimelineSim analysis.
    """
    n_tok = S * B
    TCH = min(512, n_tok)          # tokens per Gx1 chunk
    n_ch = n_tok // TCH
    steps_per_ch = TCH // B

    nc = bacc.Bacc(
        "TRN2", target_bir_lowering=False, debug=False,
        num_devices=1 if sim_mode else NC,
    )

    wr_in = nc.dram_tensor("wr", [2, 128, KT * 8 * 128], bf16, kind="ExternalInput")
    wx_in = nc.dram_tensor("wx", [2, 128, KT * 8 * 128], bf16, kind="ExternalInput")
    decw_in = nc.dram_tensor("decw", [128, KT, VS], bf16, kind="ExternalInput")
    decb_in = nc.dram_tensor("decb", [1, VS], bf16, kind="ExternalInput")
    embt_in = nc.dram_tensor("embt", [KT, 128, n_tok], bf16, kind="ExternalInput")
    h0t1_in = nc.dram_tensor("h0t1", [128, KT, B], bf16, kind="ExternalInput")
    h0t2_in = nc.dram_tensor("h0t2", [128, KT, B], bf16, kind="ExternalInput")
    c0_in = nc.dram_tensor("c0own", [2, 128, B], f32, kind="ExternalInput")
    bias_in = nc.dram_tensor("bias", [128, 8, 2], f32, kind="ExternalInput")
    probs_in = nc.dram_tensor("probs", [128, 8, 2], f32, kind="ExternalInput")

    dec_out = nc.dram_tensor("dec", [n_tok, VS], f32, kind="ExternalOutput")
    hids_out = nc.dram_tensor("hids", [2, 128, B], f32, kind="ExternalOutput")
    cells_out = nc.dram_tensor("cells", [2, 128, B], f32, kind="ExternalOutput")

    n_groups = n_tok // 128        # decoder token groups of 128
    sig_f = mybir.ActivationFunctionType.Sigmoid
    tanh_f = mybir.ActivationFunctionType.Tanh
    copy_f = mybir.ActivationFunctionType.Copy
    ident_f = mybir.ActivationFunctionType.Identity

    with tile.TileContext(nc) as tc:
        with (
            tc.tile_pool(name="wpool", bufs=1) as wpool,
            tc.tile_pool(name="state", bufs=1) as state,
            tc.tile_pool(name="ring", bufs=3) as ring,
            tc.tile_pool(name="gxring", bufs=3) as gxring,
            tc.tile_pool(name="acts", bufs=2) as acts,
            tc.tile_pool(name="o1pool", bufs=2) as o1pool,
            tc.tile_pool(name="decstg", bufs=2) as decstg,
            tc.tile_pool(name="ps1pool", bufs=2, space="PSUM") as ps1pool,
            tc.tile_pool(name="ps2pool", bufs=2, space="PSUM") as ps2pool,
            tc.tile_pool(name="bigps", bufs=4, space="PSUM") as bigps,
            tc.tile_pool(name="dram", bufs=1, space="DRAM") as dram,
            tc.tile_pool(name="agdram", bufs=3, space="DRAM") as agdram,
        ):
            # ---- resident weights & constants ----
            wr_sb = wpool.tile([128, 2, KT, 8, 128], bf16)
            nc.sync.dma_start(wr_sb[:], wr_in.ap().rearrange("l p x -> p l x"))
            wx_sb = wpool.tile([128, 2, KT, 8, 128], bf16)
            nc.sync.dma_start(wx_sb[:], wx_in.ap().rearrange("l p x -> p l x"))
            decw_sb = wpool.tile([128, KT, VS], bf16)
            nc.sync.dma_start(decw_sb[:], decw_in[:])
            decb_sb = wpool.tile([1, VS], bf16)
            nc.sync.dma_start(decb_sb[:], decb_in[:])
            bias_sb = wpool.tile([128, 8, 2], f32)
            nc.sync.dma_start(bias_sb[:], bias_in[:])
            probs_sb = wpool.tile([128, 8, 2], f32)
            nc.sync.dma_start(probs_sb[:], probs_in[:])
            ones_sb = wpool.tile([1, 128], bf16)
            nc.vector.memset(ones_sb[:], 1.0)

            c_st = state.tile([128, 2, B], f32)   # cell state, both layers
            nc.sync.dma_start(c_st[:], c0_in.ap().rearrange("l p b -> p l b"))

            gx1_dram = dram.tile([128, 8, n_tok], f32)  # [p, m, tok]

            # Warm the recurrence PSUM slots: one start=True matmul writing
            # zeros sets every has_written bit, so steady-state groups can
            # begin with start=False and accumulate onto gx data placed in
            # PSUM by DMA/ACT-copy.
            zero_rhs = wpool.tile([1, 256], bf16)
            nc.vector.memset(zero_rhs[:], 0.0)
            for pool, tagn in ((ps1pool, "ps1"), (ps2pool, "ps2")):
                for _ in range(2):
                    warm = pool.tile([128, 256], f32, tag=tagn, name="warm")
                    nc.tensor.matmul(warm[:], ones_sb[:], zero_rhs[:],
                                     start=True, stop=True)

            # ---- Gx1 chunk: batched emb @ Wx1^T with bias folded in ----
            def gx1_chunk(ch, ms):
                if ms == 0:
                    rhse = ring.tile([128, KT, TCH], bf16, tag="rhse", bufs=2)
                    gx1_chunk.rhse = rhse
                    nc.sync.dma_start(
                        rhse[:],
                        embt_in[:, :, ch * TCH:(ch + 1) * TCH].rearrange(
                            "k p t -> p k t"),
                    )
                rhse = gx1_chunk.rhse
                for m in (2 * ms, 2 * ms + 1):
                    psg = bigps.tile([128, TCH], f32, tag="bigps")
                    for k in range(KT):
                        nc.tensor.matmul(
                            psg[:], wx_sb[:, 0, k, m, :], rhse[:, k, :],
                            start=(k == 0), stop=(k == KT - 1),
                        )
                    stg = decstg.tile([128, TCH], f32, tag="gxstg", bufs=2)
                    nc.scalar.activation(stg[:], psg[:], ident_f,
                                         bias=bias_sb[:, m:m + 1, 0])
                    nc.sync.dma_start(
                        gx1_dram[:, m, ch * TCH:(ch + 1) * TCH], stg[:])

            for ms in range(4):
                gx1_chunk(0, ms)

            # ---- per-step act chain (bias already folded into gx) ----
            def act_chain(l, gsum, h_bf, emit_fp32_h):
                """gsum [128,256] f32 -> h (bf16) into h_bf [128,32];
                updates c_st[:, l, :]."""
                sig = acts.tile([128, 256], f32, tag=f"sig{l}")
                nc.scalar.activation(sig[:, 0:192], gsum[:, 0:192], sig_f)
                nc.scalar.activation(sig[:, 192:256], gsum[:, 192:256], tanh_f)
                mixed = acts.tile([128, 256], f32, tag=f"mix{l}")
                for m in range(8):
                    nc.vector.tensor_scalar_mul(
                        mixed[:, m * 32:(m + 1) * 32],
                        sig[:, m * 32:(m + 1) * 32],
                        probs_sb[:, m:m + 1, l],
                    )
                gval = acts.tile([128, 128], f32, tag=f"gval{l}")
                nc.vector.tensor_tensor(
                    gval[:, 0:96], mixed[:, 0:96], mixed[:, 96:192],
                    mybir.AluOpType.add)
                nc.vector.tensor_tensor(
                    gval[:, 96:128], mixed[:, 192:224], mixed[:, 224:256],
                    mybir.AluOpType.add)
                t1 = acts.tile([128, B], f32, tag=f"t1{l}")
                nc.vector.tensor_tensor(
                    t1[:], gval[:, 32:64], c_st[:, l, :], mybir.AluOpType.mult)
                t2 = acts.tile([128, B], f32, tag=f"t2{l}")
                nc.vector.tensor_tensor(
                    t2[:], gval[:, 0:32], gval[:, 96:128], mybir.AluOpType.mult)
                nc.vector.tensor_tensor(
                    c_st[:, l, :], t1[:], t2[:], mybir.AluOpType.add)
                tanhc = acts.tile([128, B], f32, tag=f"tanhc{l}")
                nc.scalar.activation(tanhc[:], c_st[:, l, :], tanh_f)
                nc.vector.tensor_tensor(
                    h_bf[:], gval[:, 64:96], tanhc[:], mybir.AluOpType.mult)
                if emit_fp32_h:
                    hf = acts.tile([128, B], f32, tag="hfinal")
                    nc.vector.tensor_tensor(
                        hf[:], gval[:, 64:96], tanhc[:], mybir.AluOpType.mult)
                    nc.sync.dma_start(hids_out[l], hf[:])

            def allgather(h_bf, tag):
                bounce = agdram.tile([128, B], bf16, tag=f"agin{tag}")
                nc.sync.dma_start(bounce[:], h_bf[:])
                ag = agdram.tile([128 * NC, B], bf16, tag=f"agout{tag}",
                                 addr_space="Local" if sim_mode else "Shared")
                if sim_mode:
                    nc.sync.dma_start(ag[0:128, :], bounce[:])
                else:
                    nc.gpsimd.collective_compute(
                        "AllGather", mybir.AluOpType.bypass,
                        replica_groups=[list(range(NC))],
                        ins=[bounce.opt()], outs=[ag.opt()],
                    )
                return ag

            def decode_quarter(g, q, o1g):
                w = VS // 8
                for ch in range(2):
                    col0 = (q * 2 + ch) * w
                    psd = bigps.tile([128, w], f32, tag="bigps")
                    for k in range(KT):
                        nc.tensor.matmul(
                            psd[:, 0:w], o1g[:, k, :],
                            decw_sb[:, k, col0:col0 + w],
                            start=(k == 0), stop=False,
                        )
                    nc.tensor.matmul(
                        psd[:, 0:w], ones_sb[:], decb_sb[:, col0:col0 + w],
                        start=False, stop=True,
                    )
                    stg = decstg.tile([128, w], f32, tag="decstg")
                    nc.scalar.activation(stg[:, 0:w], psd[:, 0:w], copy_f)
                    nc.sync.dma_start(
                        dec_out[g * 128:(g + 1) * 128, col0:col0 + w],
                        stg[:, 0:w])

            o0_tiles = {}
            o1_tiles = {}
            gx2_tiles = {}
            ag1 = ag2 = None
            for s in range(S + D2 + 1):
                # ---- gathered h^T tiles for this superstep ----
                if s == 0:
                    ht1 = ring.tile([128, KT, B], bf16, tag="ht1")
                    nc.sync.dma_start(ht1[:], h0t1_in[:])
                elif s <= S:
                    ht1 = ring.tile([128, KT, B], bf16, tag="ht1")
                    nc.sync.dma_start(
                        ht1[:], ag1.rearrange("(k p) b -> p k b", p=128))
                if s == D2:
                    ht2 = ring.tile([128, KT, B], bf16, tag="ht2")
                    nc.sync.dma_start(ht2[:], h0t2_in[:])
                elif s >= D2 + 1:
                    ht2 = ring.tile([128, KT, B], bf16, tag="ht2")
                    nc.sync.dma_start(
                        ht2[:], ag2.rearrange("(k p) b -> p k b", p=128))

                # ---- o0 history (h1) for Gx2 batching ----
                if 1 <= s <= S:
                    t1s = s - 1
                    g, slot = t1s // 4, t1s % 4
                    if slot == 0:
                        o0g_t = o1pool.tile([128, KT, 128], bf16, tag="o0g", name="o0g_t")
                        o0_tiles[g] = o0g_t
                    nc.vector.tensor_copy(
                        o0_tiles[g][:, :, slot * 32:(slot + 1) * 32], ht1[:])
                # ---- o1 history (h2) for the decoder ----
                if s >= D2 + 1:
                    t2s = s - D2 - 1
                    g, slot = t2s // 4, t2s % 4
                    if slot == 0:
                        o1g_t = o1pool.tile([128, KT, 128], bf16, tag="o1g", name="o1g_t")
                        o1_tiles[g] = o1g_t
                    nc.vector.tensor_copy(
                        o1_tiles[g][:, :, slot * 32:(slot + 1) * 32], ht2[:])

                # ---- interleaved Gx1 chunks ----
                if s % 16 < 4 and s // 16 + 1 < n_ch:
                    gx1_chunk(s // 16 + 1, s % 16)

                # ---- Gx2 batch for group g at s = 4g+5 ----
                if s >= 5 and (s - 5) % 4 == 0 and (s - 5) // 4 < S // 4:
                    g = (s - 5) // 4
                    gx2g = gxring.tile([128, 8, 128], f32, tag="gx2g", bufs=2)
                    gx2_tiles[g] = gx2g  # noqa
                    for half in range(2):
                        psb = bigps.tile([128, 512], f32, tag="bigps")
                        for mi in range(4):
                            m = half * 4 + mi
                            for k in range(KT):
                                nc.tensor.matmul(
                                    psb[:, mi * 128:(mi + 1) * 128],
                                    wx_sb[:, 1, k, m, :],
                                    o0_tiles[g][:, k, :],
                                    start=(k == 0), stop=(k == KT - 1),
                                )
                        for mi in range(4):
                            m = half * 4 + mi
                            nc.scalar.activation(
                                gx2g[:, m, :],
                                psb[:, mi * 128:(mi + 1) * 128],
                                ident_f, bias=bias_sb[:, m:m + 1, 1])

                # ---- L1 step s ----
                if s < S:
                    gx = gxring.tile([128, 8, B], f32, tag="gx")
                    nc.sync.dma_start(
                        gx[:], gx1_dram[:, :, s * B:(s + 1) * B])
                    ps1 = ps1pool.tile([128, 256], f32, tag="ps1")
                    nc.scalar.copy(
                        ps1.rearrange("p (m b) -> p m b", b=B), gx[:])
                    for m in range(8):
                        for k in range(KT):
                            nc.tensor.matmul(
                                ps1[:, m * 32:(m + 1) * 32],
                                wr_sb[:, 0, k, m, :], ht1[:, k, :],
                                start=False, stop=(k == KT - 1),
                                skip_group_check=True,
                            )
                    h1_bf = acts.tile([128, B], bf16, tag="h1bf")
                    act_chain(0, ps1, h1_bf, emit_fp32_h=(s == S - 1))
                    ag1 = allgather(h1_bf, 1)

                # ---- L2 step s - D2 ----
                if D2 <= s < S + D2:
                    t2 = s - D2
                    ps2 = ps2pool.tile([128, 256], f32, tag="ps2")
                    nc.scalar.copy(
                        ps2.rearrange("p (m b) -> p m b", b=B),
                        gx2_tiles[t2 // 4][:, :, (t2 % 4) * 32:(t2 % 4 + 1) * 32])
                    for m in range(8):
                        for k in range(KT):
                            nc.tensor.matmul(
                                ps2[:, m * 32:(m + 1) * 32],
                                wr_sb[:, 1, k, m, :], ht2[:, k, :],
                                start=False, stop=(k == KT - 1),
                                skip_group_check=True,
                            )
                    h2_bf = acts.tile([128, B], bf16, tag="h2bf")
                    act_chain(1, ps2, h2_bf, emit_fp32_h=(s == S + D2 - 1))
                    ag2 = allgather(h2_bf, 2)

                # ---- decoder: one quarter-group per superstep once ready ----
                for q in range(4):
                    gq, rem = divmod(s - (D2 + 4 + q), 4)
                    if rem == 0 and 0 <= gq < n_groups - 1:
                        decode_quarter(gq, q, o1_tiles[gq])

            # ---- epilogue: final decoder group ----
            gq = n_groups - 1
            for q in range(4):
                decode_quarter(gq, q, o1_tiles[gq])

            # ---- state outputs ----
            for l in range(2):
                nc.sync.dma_start(cells_out[l], c_st[:, l, :])

    nc.compile()
    return nc


_prog_cache = {}


def _get_program(S):
    if S not in _prog_cache:
        _prog_cache[S] = build_program(S)
    return _prog_cache[S]


def _prep_inputs(S, x, h0, c0, emb_W, dec_W, dec_b, W_ih, b_ih, W_hh, Wb, bb, mix):
    """Host-side prep: returns in_maps (list of 8 dicts)."""
    n_tok = S * B
    x = np.asarray(x).reshape(-1)
    emb = np.asarray(emb_W)[x]                      # [n_tok, NINP]
    embt = np.ascontiguousarray(emb.T).astype(BF16).reshape(KT, 128, n_tok)

    mix = np.asarray(mix)
    probs = np.exp(mix - np.max(mix, -1, keepdims=True))
    probs = (probs / probs.sum(-1, keepdims=True)).astype(np.float32)  # [2,4,2]

    Wb = np.asarray(Wb)
    W_ih = np.asarray(W_ih)
    W_hh = np.asarray(W_hh)
    b_ih = np.asarray(b_ih)
    bb = np.asarray(bb)
    h0 = np.asarray(h0)
    c0 = np.asarray(c0)
    dec_W = np.asarray(dec_W)
    dec_b = np.asarray(dec_b)

    # per-layer big matrices, rows ordered by m-tile blocks of H rows
    def stack_rows(l, part):  # part: 0 = x-side [.., :NINP], 1 = h-side
        blocks = []
        for t, is_bay in M_ORDER:
            if is_bay:
                w = Wb[l][t][:, NINP:] if part else Wb[l][t][:, :NINP]
            else:
                w = W_hh[l][t * H:(t + 1) * H] if part else W_ih[l][t * H:(t + 1) * H]
            blocks.append(w)
        return np.stack(blocks)                      # [8, H, 1024]

    wx_all = [stack_rows(l, 0) for l in range(2)]
    wr_all = [stack_rows(l, 1) for l in range(2)]

    def core_w(wlist, c):
        # -> [2, 128, KT*8*128]: [l, p, k*1024 + m*128 + j]
        out = np.empty((2, 128, KT * 8 * 128), BF16)
        for l in range(2):
            a = wlist[l][:, c * HS:(c + 1) * HS, :]   # [8m, 128j, 1024K]
            a = a.reshape(8, HS, KT, 128)             # [m, j, k, p]
            out[l] = a.transpose(3, 2, 0, 1).reshape(128, -1).astype(BF16)
        return out

    def core_bias(c):
        out = np.zeros((128, 8, 2), np.float32)
        for l in range(2):
            for m, (t, is_bay) in enumerate(M_ORDER):
                if is_bay:
                    out[:, m, l] = bb[l][t][c * HS:(c + 1) * HS]
                else:
                    out[:, m, l] = 2.0 * b_ih[l][t * H + c * HS:t * H + (c + 1) * HS]
        return out

    def core_probs():
        out = np.zeros((128, 8, 2), np.float32)
        for l in range(2):
            for m, (t, is_bay) in enumerate(M_ORDER):
                out[:, m, l] = probs[l, t, 1 if is_bay else 0]
        return out

    h0t1 = h0[0].T.reshape(KT, 128, B).transpose(1, 0, 2).astype(BF16)
    h0t2 = h0[1].T.reshape(KT, 128, B).transpose(1, 0, 2).astype(BF16)

    probs_arr = core_probs()
    in_maps = []
    for c in range(NC):
        dwt = dec_W[c * VS:(c + 1) * VS].T           # [1024, VS]
        decw = np.ascontiguousarray(dwt).astype(BF16).reshape(KT, 128, VS)
        decw = np.ascontiguousarray(decw.transpose(1, 0, 2))
        in_maps.append(
            dict(
                wr=core_w(wr_all, c),
                wx=core_w(wx_all, c),
                decw=decw,
                decb=dec_b[c * VS:(c + 1) * VS].reshape(1, VS).astype(BF16),
                embt=embt,
                h0t1=np.ascontiguousarray(h0t1),
                h0t2=np.ascontiguousarray(h0t2),
                c0own=np.ascontiguousarray(
                    np.stack([c0[l][:, c * HS:(c + 1) * HS].T for l in range(2)])
                ).astype(np.float32),
                bias=core_bias(c),
                probs=probs_arr,
            )
        )
    return in_maps


def run(S, inputs):
    nc = _get_program(S)
    in_maps = _prep_inputs(S, **inputs)
    res = run_bass_kernel_spmd(nc, in_maps, core_ids=list(range(NC)))
    dec = np.concatenate(
        [r["dec"].reshape(S, B, VS) for r in res.results], axis=2)
    hids = np.concatenate(
        [r["hids"].transpose(0, 2, 1) for r in res.results], axis=2)
    cells = np.concatenate(
        [r["cells"].transpose(0, 2, 1) for r in res.results], axis=2)
    return dec, hids, cells


def kernel(**inputs):
    S = np.asarray(inputs["x"]).shape[0]
    return run(S, inputs)
